# revision 4
# baseline (speedup 1.0000x reference)
"""Trainium2 kernel for nn_BCD_GLPLinearA_58342835748977.

Strategy: the vocab projection logits = final @ sign_w.T + sign_b with
output [2, 2048, 32000] f32 (~524 MB) dominates both memory traffic and
FLOPs; everything upstream of `final` touches < 1% of the bytes.  So the
host computes the small prologue (wells / resonant-tunneling attention)
in numpy, and the 8 NeuronCores each compute a 4000-wide vocab shard of
the projection (sign_w/sign_b/logits tensor-sharded over vocab; the
[65, 4096] activation matrix is replicated).  The bias is folded into
the matmul by augmenting the contraction dim with a ones row (K=65).
"""

import math
import threading

import numpy as np

import concourse.bacc as bacc
import concourse.mybir as mybir
from concourse.bass_utils import run_bass_kernel_spmd
from concourse.tile import TileContext

# Problem constants (hardcoded per contest contract).
B, S, D, V = 2, 2048, 64, 32000
SCALES = [2, 3, 5, 8, 13, 21]
MAXN = [min(s, 8) for s in SCALES]  # [2,3,5,8,8,8]
TEMP = 0.1
RES = 256
N_CORES = 8
VS = V // N_CORES          # 4000 vocab per core
M = B * S                  # 4096 token rows
K = D + 1                  # 65: contraction dim with bias row folded in
N_TILE = 500               # free-dim tile; 500 f32 fits one PSUM bank
M_TILE = 128


def _hermite_basis(max_n, sigma):
    # float64 math then f32 cast, exactly like the reference.
    x = np.linspace(-3.0, 3.0, RES)
    xi = x / (sigma * np.sqrt(2.0))
    Hs = [np.ones_like(xi)]
    if max_n > 1:
        Hs.append(2.0 * xi)
    for n in range(1, max_n - 1):
        Hs.append(2.0 * xi * Hs[n] - 2.0 * n * Hs[n - 1])
    env = np.exp(-xi ** 2 / 2.0)
    rows = [((2.0 ** n * math.factorial(n) * np.sqrt(np.pi)) ** -0.5) * Hs[n] * env
            for n in range(max_n)]
    return np.stack(rows).astype(np.float32)  # [max_n, RES]


def _softmax(z, axis):
    z = z - z.max(axis=axis, keepdims=True)
    e = np.exp(z)
    return e / e.sum(axis=axis, keepdims=True)


_nc_cache = {}
_nc_lock = threading.Lock()


def _build_kernel():
    """Per-core SPMD program: out[4096, 4000] = a[65, 4096].T @ w[65, 4000]."""
    F32 = mybir.dt.float32
    F32R = mybir.dt.float32r

    nc = bacc.Bacc(target_bir_lowering=False)
    a_ext = nc.declare_dram_parameter("a", [K, M], F32R, isOutput=False)
    w_ext = nc.declare_dram_parameter("w", [K, VS], F32R, isOutput=False)
    out_ext = nc.declare_dram_parameter("out", [M, VS], F32, isOutput=True)

    with TileContext(nc) as tc:
        with (
            tc.tile_pool(name="a_pool", bufs=1) as a_pool,
            tc.tile_pool(name="w_pool", bufs=1) as w_pool,
            tc.tile_pool(name="psum", bufs=6, space="PSUM") as psum_pool,
            tc.tile_pool(name="o_pool", bufs=3) as o_pool,
        ):
            a_t = a_pool.tile([K, M], F32R)
            w_t = w_pool.tile([K, VS], F32R)
            nc.sync.dma_start(out=a_t[:], in_=a_ext[:])
            nc.sync.dma_start(out=w_t[:], in_=w_ext[:])
            for m in range(M // M_TILE):
                o_t = o_pool.tile([M_TILE, VS], F32)
                for n in range(VS // N_TILE):
                    ps = psum_pool.tile([M_TILE, N_TILE], F32)
                    nc.tensor.matmul(
                        ps[:],
                        lhsT=a_t[:, m * M_TILE:(m + 1) * M_TILE],
                        rhs=w_t[:, n * N_TILE:(n + 1) * N_TILE],
                        start=True, stop=True,
                    )
                    nc.vector.tensor_copy(o_t[:, n * N_TILE:(n + 1) * N_TILE], ps[:])
                nc.sync.dma_start(
                    out=out_ext[m * M_TILE:(m + 1) * M_TILE, :], in_=o_t[:])
    nc.finalize()
    return nc


def _get_kernel():
    with _nc_lock:
        if "nc" not in _nc_cache:
            _nc_cache["nc"] = _build_kernel()
        return _nc_cache["nc"]


def _prologue(sign_ids, embedding, occ, S_matrix, res_energy, res_width):
    x = embedding[sign_ids]                    # [B, S, D]
    seq_mean = x.mean(axis=1)                  # [B, D]
    pos = np.linspace(-1.0, 1.0, S).astype(np.float32)
    idx = np.clip(((pos + np.float32(1.0)) / np.float32(2.0)
                   * np.float32(255.0)).astype(np.int32), 0, 255)

    wells = []
    for i, (scale, mn) in enumerate(zip(SCALES, MAXN)):
        basis = _hermite_basis(mn, scale / 5.0)[:, idx]            # [mn, S]
        w, b = occ[i]
        amps = _softmax(seq_mean @ w.T + b, axis=-1)               # [B, mn]
        wf = amps @ basis                                          # [B, S]
        wells.append(wf[:, :, None] * seq_mean[:, None, :])        # [B, S, D]
    ws = np.stack(wells, axis=1)                                   # [B, 6, S, D]

    qE = ws.mean(axis=(1, 2))                                      # [B, D]
    G = np.abs(res_width)                                          # [6, D]
    amp = G / np.sqrt((qE[:, None, :] - res_energy[None]) ** 2
                      + (G / np.float32(2.0)) ** 2 + np.float32(1e-8))
    Smix = _softmax(S_matrix / np.float32(TEMP), axis=1)           # [6, 6, D]
    mixed = np.einsum("ijh,bjsh->bish", Smix, ws)                  # [B, 6, S, D]
    tunneled = mixed * amp[:, :, None, :]

    final = tunneled.sum(axis=1)                                   # [B, S, D]
    tablet = final.mean(axis=1)                                    # [B, D]
    strength = np.float32(np.std(amp, axis=1, ddof=1).mean())
    return final, tablet, amp, strength


def _make_device_inputs(final, sign_w, sign_b):
    f32 = np.float32
    a_aug = np.empty((K, M), f32)
    a_aug[:D, :] = final.reshape(M, D).T
    a_aug[D, :] = 1.0
    w_aug = np.empty((K, V), f32)
    w_aug[:D, :] = sign_w.T
    w_aug[D, :] = sign_b
    return [{"a": a_aug,
             "w": np.ascontiguousarray(w_aug[:, c * VS:(c + 1) * VS])}
            for c in range(N_CORES)]


def kernel(sign_ids, embedding, w_occ_0, b_occ_0, w_occ_1, b_occ_1, w_occ_2,
           b_occ_2, w_occ_3, b_occ_3, w_occ_4, b_occ_4, w_occ_5, b_occ_5,
           S_matrix, res_energy, res_width, sign_w, sign_b, geo_w, geo_b):
    f32 = np.float32
    sign_ids = np.asarray(sign_ids)
    embedding = np.ascontiguousarray(embedding, dtype=f32)
    occ = [(np.asarray(w_occ_0, f32), np.asarray(b_occ_0, f32)),
           (np.asarray(w_occ_1, f32), np.asarray(b_occ_1, f32)),
           (np.asarray(w_occ_2, f32), np.asarray(b_occ_2, f32)),
           (np.asarray(w_occ_3, f32), np.asarray(b_occ_3, f32)),
           (np.asarray(w_occ_4, f32), np.asarray(b_occ_4, f32)),
           (np.asarray(w_occ_5, f32), np.asarray(b_occ_5, f32))]
    S_matrix = np.asarray(S_matrix, f32)
    res_energy = np.asarray(res_energy, f32)
    res_width = np.asarray(res_width, f32)
    sign_w = np.asarray(sign_w, f32)
    sign_b = np.asarray(sign_b, f32)
    geo_w = np.asarray(geo_w, f32)
    geo_b = np.asarray(geo_b, f32)

    final, tablet, amp, strength = _prologue(
        sign_ids, embedding, occ, S_matrix, res_energy, res_width)
    geometry = tablet @ geo_w.T + geo_b                            # [B, 3]

    # Device part: vocab-sharded projection with bias folded into K.
    nc = _get_kernel()
    in_maps = _make_device_inputs(final, sign_w, sign_b)
    res = run_bass_kernel_spmd(nc, in_maps, list(range(N_CORES)))

    logits = np.empty((M, V), f32)
    for c in range(N_CORES):
        logits[:, c * VS:(c + 1) * VS] = res.results[c]["out"]
    logits = logits.reshape(B, S, V)

    return (tablet, logits, geometry, amp, strength)
